# revision 1
# baseline (speedup 1.0000x reference)
"""Trainium2 Bass kernel for CapsuleLayer dynamic routing.

Problem: inputs [64, 2048, 8] f32, W [32, 2048, 16, 8] f32
  inputs_hat[b,n,i,e] = sum_d inputs[b,i,d] * W[n,i,e,d]
  3 routing iterations (softmax over n, weighted sums over i, squash)
  -> outputs [64, 32, 16] f32

Strategy: data-parallel over batch across 8 cores (8 batches each, W
replicated).  Per core:
  Phase 1: stream W (pre-cast bf16, pre-tiled on host into 128 chunks of
    [128=(i16,d8), 512=(e,n)], DMA'd 4 chunks at a time) and compute
    inputs_hat via block-diagonal-inputs matmuls on the PE; the uniform-c
    s_0 = sum_{i,d} x*W accumulates via a second matmul per chunk with
    lhs = x chunk [128, 8] (independent of the PSUM evacuation, so the
    PE runs back-to-back and reaches its full p-state).  PSUM->SBUF bf16
    evacuation alternates ACT/Pool, keeping DVE free.
  Phase 2 (routing tail, on-chip): ih[p=(i16,b8), f=(k,e,n)] so the
    weighted i-reduction s_r = sum_i c*ih runs as contiguous-rhs
    delta-mask PE matmuls accumulating into one PSUM bank, and every big
    DVE multiply hits the 2x bf16 mode (innermost n, step 1; broadcasts
    on outer/middle dims).  b-update via DVE mul + contiguous e-halving
    tree.  softmax over n on ACT exp + DVE reduce/reciprocal.  squash via
    Ln/Exp (one ACT table set, no sqrt table switch).  v is replicated
    to 128 partitions via a PE broadcast matmul (rep-mask lhs) + ACT
    evac instead of 16 small DMAs.
"""

import numpy as np

B, I, DI = 64, 2048, 8
N, DO = 32, 16
CORES = 8
BL = B // CORES  # 8 batches per core
KC = 128         # i-chunks
ISUB = 16        # i per chunk
FNE = N * DO     # 512
KB = 16          # chunks per tail block
NBLK = KC // KB  # 8 tail blocks
WG = 2           # W chunks per DMA
EPS = 1e-7

_CACHE = {}


def _patch_tile_tail_barrier():
    """The walrus build in this container rejects >1 sync-wait on the Tile
    tail Drain.  Replace the multi-wait drain with one wait_ge per
    outstanding semaphore (SP executes them in order), then a bare drain."""
    import concourse.tile as tile

    if getattr(tile.TileContext, "_ant_split_drain_patch", False):
        return

    def _drain_and_barrier(self, tick_clock, wait_clock):
        gc = tick_clock.global_clock
        ticks = eval(repr(gc).replace("VectorClock(", "").rstrip(")"))
        for idx, sem in sorted(self.sems.allocated().items()):
            if idx < len(ticks) and ticks[idx] > 0:
                mult = 16 if idx >= 11 else 1
                self.nc.sync.wait_ge(sem, ticks[idx] * mult)
        self.nc.sync.drain()
        self.nc.all_engine_barrier()
        popped = self.nc._tile_sem_poison_stack.pop()
        assert popped is self._sem_poison
        self.nc.clear_and_free_semaphores(list(self.sems.allocated().values()))

    tile.TileContext._drain_and_barrier = _drain_and_barrier
    tile.TileContext._ant_split_drain_patch = True


def _split_multi_waits(bir_bytes):
    """This container's walrus build allows only one sync-wait per
    instruction.  Hoist extra semaphore waits onto preceding wait-only
    EventSemaphore instructions on the same engine (engines execute their
    stream in order, so semantics are preserved)."""
    import json

    d = json.loads(bir_bytes)
    ctr = 0
    for f in d["functions"]:
        for blk in f["blocks"]:
            out = []
            for ins in blk["instructions"]:
                waits = ins.get("sync_info", {}).get("on_wait", [])
                if len(waits) > 1:
                    for w in waits[:-1]:
                        ctr += 1
                        out.append({
                            "debug": ins.get("debug", 0),
                            "engine": ins["engine"],
                            "ins": [],
                            "name": f"antwaitsplit-{ctr}",
                            "opcode": "EventSemaphore",
                            "outs": [],
                            "sync_info": {"on_update": [], "on_wait": [w]},
                        })
                    ins["sync_info"]["on_wait"] = [waits[-1]]
                out.append(ins)
            blk["instructions"] = out
    return json.dumps(d).encode()


def _patch_compile_split_waits():
    from concourse import bass2jax, bass_utils

    if getattr(bass_utils, "_ant_split_waits_patch", False):
        return
    orig = bass_utils.compile_bir_kernel

    def patched(bir_json, tmpdir, neff_name="file.neff"):
        return orig(_split_multi_waits(bir_json), tmpdir, neff_name)

    bass_utils.compile_bir_kernel = patched
    bass_utils._ant_split_waits_patch = True
    if getattr(bass2jax, "compile_bir_kernel", None) is orig:
        bass2jax.compile_bir_kernel = patched


def _build_nc():
    import concourse.bass as bass
    import concourse.tile as tile
    from concourse import mybir

    _patch_tile_tail_barrier()
    _patch_compile_split_waits()

    f32 = mybir.dt.float32
    bf16 = mybir.dt.bfloat16
    AF = mybir.ActivationFunctionType
    OP = mybir.AluOpType
    AX = mybir.AxisListType

    nc = bass.Bass(target_bir_lowering=False)

    wprep = nc.dram_tensor("wprep", [KC, 128, FNE], bf16, kind="ExternalInput")
    xprep = nc.dram_tensor("xprep", [128, KC, BL], bf16, kind="ExternalInput")
    bmask = nc.dram_tensor("bmask", [128, 128], bf16, kind="ExternalInput")
    dmask = nc.dram_tensor("dmask", [128, BL], bf16, kind="ExternalInput")
    rmask = nc.dram_tensor("rmask", [BL, 128], bf16, kind="ExternalInput")
    out_d = nc.dram_tensor("out", [BL, FNE], f32, kind="ExternalOutput")

    dma_engines = [nc.sync, nc.gpsimd]

    with tile.TileContext(nc) as tc:
        with (
            tc.tile_pool(name="big", bufs=1) as big,
            tc.tile_pool(name="spsum", bufs=1, space="PSUM") as spp,
            tc.tile_pool(name="small", bufs=1) as small,
            tc.tile_pool(name="consts", bufs=1) as consts,
        ):
            # persistent tensors; ih free dims = (k, e, n)
            ih = big.tile([128, KC, DO, N], bf16, name="ih")
            b_acc = big.tile([128, KC, N], bf16, name="b_acc")
            dm = consts.tile([128, BL], bf16, name="dm")
            nc.sync.dma_start(dm[:], dmask[:])
            bm = consts.tile([128, 128], bf16, name="bm")
            nc.sync.dma_start(bm[:], bmask[:])
            rm = consts.tile([BL, 128], bf16, name="rm")
            nc.sync.dma_start(rm[:], rmask[:])
            xp = consts.tile([128, KC, BL], bf16, name="xp")
            nc.gpsimd.dma_start(xp[:], xprep[:])
            epsb = consts.tile([BL, 1], f32, name="epsb")
            nc.vector.memset(epsb[:], EPS)

            # ---------------- Phase 1: W stream ----------------
            # s0 = sum_i ih accumulates on DVE (8 k-slots, bf16) chasing the
            # evac stream; one final dmask matmul folds (i16, slots) in f32.
            s0 = spp.tile([BL, FNE], f32, name="s0")
            SLOTS = 8
            with (
                tc.tile_pool(name="wstream", bufs=24) as wpool,
                tc.tile_pool(name="bstream", bufs=6) as bpool,
                tc.tile_pool(name="mmpsum", bufs=6, space="PSUM") as mmp,
                tc.tile_pool(name="s0pool", bufs=1) as s0pool,
            ):
                acc = s0pool.tile([128, SLOTS, DO, N], bf16, name="s0acc")
                for g in range(KC // WG):
                    eng = dma_engines[1] if g % 4 == 3 else dma_engines[0]
                    wc = wpool.tile([128, WG, FNE], bf16, name="wc")
                    eng.dma_start(
                        wc[:],
                        wprep[g * WG:(g + 1) * WG].rearrange("k p f -> p k f"))
                    for j in range(WG):
                        k = g * WG + j
                        wcj = wc[:, j, :]
                        bp = bpool.tile([128, 128], bf16, name="bp")
                        beng = nc.gpsimd if k % 4 == 3 else nc.vector
                        beng.tensor_mul(
                            bp[:].rearrange("p (i b) -> p i b", b=BL),
                            bm[:].rearrange("p (i b) -> p i b", b=BL),
                            xp[:, k, None, :].broadcast_to([128, ISUB, BL]))
                        ps = mmp.tile([128, FNE], f32, name="ps")
                        nc.tensor.matmul(ps[:], bp[:], wcj, start=True,
                                         stop=True, skip_group_check=True)
                        # evacuate PSUM [(i,b),(e,n)] -> ih[p, k, e, n] bf16
                        dst = ih[:, k, :, :].rearrange("p e n -> p (e n)")
                        if k % 8 == 7:
                            nc.vector.tensor_copy(dst, ps[:])
                        else:
                            nc.scalar.copy(dst, ps[:])
                    # k-slot accumulation once a full slab of 8 chunks landed
                    kk = (g + 1) * WG
                    if kk % SLOTS == 0:
                        t = kk // SLOTS - 1
                        slab = ih[:, t * SLOTS:(t + 1) * SLOTS, :, :]
                        if t == 0:
                            nc.vector.tensor_copy(acc[:], slab)
                        else:
                            nc.vector.tensor_add(acc[:], acc[:], slab)
                # fold slots and reduce (i16, b8) partitions via dmask matmul
                h = SLOTS
                while h > 1:
                    h //= 2
                    nc.vector.tensor_add(acc[:, 0:h], acc[:, 0:h],
                                         acc[:, h:2 * h])
                nc.tensor.matmul(
                    s0[:], dm[:],
                    acc[:, 0, :, :].rearrange("p e n -> p (e n)"),
                    start=True, stop=True, skip_group_check=True)

            # ---------------- squash helper (cols are (e, n)) ---------
            def squash(s_psum, r):
                s_sb = small.tile([BL, FNE], f32, name="s_sb", tag="s_sb")
                scale0 = (1.0 / N) if r == 0 else 1.0
                nc.scalar.mul(s_sb[:], s_psum[:], scale0)
                sqv = small.tile([BL, FNE], f32, name="sqv", tag="sqv")
                nc.scalar.activation(sqv[:], s_psum[:], AF.Square, scale=scale0)
                s2 = small.tile([BL, N], f32, name="s2", tag="s2")
                nc.vector.tensor_reduce(
                    s2[:], sqv[:].rearrange("b (e n) -> b n e", e=DO),
                    axis=AX.X, op=OP.add)
                l1 = small.tile([BL, N], f32, name="l1", tag="l1")
                nc.scalar.activation(l1[:], s2[:], AF.Ln, bias=1.0)
                l2 = small.tile([BL, N], f32, name="l2", tag="l2")
                nc.scalar.activation(l2[:], s2[:], AF.Ln, bias=epsb[:])
                tt = small.tile([BL, N], f32, name="tt", tag="tt")
                nc.vector.scalar_tensor_tensor(
                    tt[:], l2[:], -0.5, l1[:],
                    op0=OP.mult, op1=OP.subtract)
                sc = small.tile([BL, N], f32, name="sc", tag="sc")
                nc.scalar.activation(sc[:], tt[:], AF.Exp)
                nc.vector.tensor_mul(sc[:], sc[:], s2[:])
                vdt = f32 if r == 2 else bf16
                v_f = small.tile([BL, DO, N], vdt, name="v_f", tag="v_f")
                nc.vector.tensor_tensor(
                    v_f[:], s_sb[:].rearrange("b (e n) -> b e n", e=DO),
                    sc[:, None, :].broadcast_to([BL, DO, N]), op=OP.mult)
                return v_f

            v_f = squash(s0, 0)

            # ---------------- routing iterations ----------------
            with (
                tc.tile_pool(name="blk", bufs=2) as blkpool,
                tc.tile_pool(name="vpsum", bufs=1, space="PSUM") as vpp,
            ):
              for r in (1, 2):
                  # replicate v to 128 partitions via PE broadcast matmul
                  vps = vpp.tile([128, FNE], f32, name="vps", tag="vps")
                  nc.tensor.matmul(vps[:], rm[:],
                                   v_f[:].rearrange("b e n -> b (e n)"),
                                   start=True, stop=True, skip_group_check=True)
                  vrep = small.tile([128, DO, N], bf16, name="vrep", tag="vrep")
                  nc.scalar.copy(vrep[:].rearrange("p e n -> p (e n)"), vps[:])

                  s_ps = spp.tile([BL, FNE], f32, name="s_ps", tag="s_ps")
                  BLKS = [18] * 7 + [2]
                  OFFS = [sum(BLKS[:i]) for i in range(len(BLKS))]
                  KBM = max(BLKS)

                  def p2chain(blk):
                      # b-update for blk: p2 = ih * v with e OUTER in the
                      # free layout (contiguous tree levels).
                      k0, kb = OFFS[blk], BLKS[blk]
                      ihb_e = ih[:, k0:k0 + kb, :, :].rearrange(
                          "p kb e n -> p e kb n")
                      p2f = blkpool.tile([128, DO, KBM, N], bf16, name="p2",
                                         tag="pp", bufs=3)
                      p2 = p2f[:, :, 0:kb, :]
                      vb = vrep[:, :, None, :].broadcast_to([128, DO, kb, N])
                      nc.vector.tensor_tensor(p2, ihb_e, vb, op=OP.mult)
                      h = DO
                      while h > 2:
                          h //= 2
                          nc.vector.tensor_add(
                              p2[:, 0:h], p2[:, 0:h], p2[:, h:2 * h])
                      bslc = b_acc[:, k0:k0 + kb, :]
                      if r == 1:
                          nc.vector.tensor_add(bslc, p2[:, 0, :, :],
                                               p2[:, 1, :, :])
                      else:
                          nc.vector.tensor_add(p2[:, 0, :, :], p2[:, 0, :, :],
                                               p2[:, 1, :, :])
                          nc.vector.tensor_add(bslc, bslc, p2[:, 0, :, :])
                      return bslc

                  for blk in range(len(BLKS)):
                      k0, kb = OFFS[blk], BLKS[blk]
                      ihb = ih[:, k0:k0 + kb, :, :]  # [p,kb,e,n]
                      bslc = p2chain(blk)
                      # --- softmax over n ---
                      ebf = blkpool.tile([128, KBM, N], bf16, name="eb",
                                         tag="eb")
                      eb = ebf[:, 0:kb, :]
                      nc.scalar.activation(eb, bslc, AF.Exp)
                      nsf = blkpool.tile([128, KBM], f32, name="ns", tag="ns")
                      ns = nsf[:, 0:kb]
                      nc.vector.tensor_reduce(ns, eb, axis=AX.X, op=OP.add)
                      recf = blkpool.tile([128, KBM], f32, name="rec",
                                          tag="rec")
                      rec = recf[:, 0:kb]
                      nc.vector.reciprocal(rec, ns)
                      # fold 1/Z into the dmask lhs: zm[p, kb, b] = dm * rec
                      zmf = blkpool.tile([128, KBM, BL], bf16, name="zm",
                                         tag="zm")
                      zm = zmf[:, 0:kb, :]
                      nc.vector.tensor_tensor(
                          zm, dm[:, None, :].broadcast_to([128, kb, BL]),
                          rec[:, :, None].broadcast_to([128, kb, BL]),
                          op=OP.mult)
                      # --- s partial: p3 = ih * exp(b), PE reduces over i
                      #     with the 1/Z-scaled lhs ---
                      p3f = blkpool.tile([128, KBM, DO, N], bf16, name="p3",
                                         tag="pp", bufs=3)
                      p3 = p3f[:, 0:kb, :, :]
                      ebb = eb[:, :, None, :].broadcast_to([128, kb, DO, N])
                      nc.vector.tensor_tensor(p3, ihb, ebb, op=OP.mult)
                      for kk in range(kb):
                          k = k0 + kk
                          nc.tensor.matmul(
                              s_ps[:], zm[:, kk, :],
                              p3[:, kk, :, :].rearrange("p e n -> p (e n)"),
                              start=(k == 0), stop=(k == KC - 1),
                              skip_group_check=True)

                  v_f = squash(s_ps, r)

            nc.sync.dma_start(out_d[:], v_f[:].rearrange("b e n -> b (e n)"))

    return nc


def _host_prep(inputs, W):
    import ml_dtypes
    bf = ml_dtypes.bfloat16

    # W_prep [128, 128, 512]: [k, (i16,d8), (e,n)]
    wt = np.transpose(W, (1, 3, 2, 0))  # [i, d, e, n]
    wflat = wt.reshape(KC, ISUB * DI, DO * N)
    wprep = np.ascontiguousarray(wflat).astype(bf)

    # delta mask [128=(i16,b8), 8]; blockdiag mask [128, 128]
    dmask = np.tile(np.eye(BL, dtype=np.float32), (ISUB, 1)).astype(bf)
    # rows (i16, d8), cols (i16, b8): 1 iff row-i == col-i
    bmask = np.kron(np.eye(ISUB, dtype=np.float32),
                    np.ones((DI, BL), dtype=np.float32)).astype(bf)
    # v replication mask: out row (g16, b8) <- v row b
    rmask = np.tile(np.eye(BL, dtype=np.float32), (1, ISUB)).astype(bf)

    in_maps = []
    for c in range(CORES):
        ic = inputs[c * BL:(c + 1) * BL]  # [8, 2048, 8]
        base = np.transpose(ic, (1, 2, 0)).reshape(KC, ISUB, DI, BL)
        xprep = np.ascontiguousarray(
            base.transpose(1, 2, 0, 3).reshape(ISUB * DI, KC, BL)).astype(bf)
        in_maps.append({"wprep": wprep, "xprep": xprep, "bmask": bmask,
                        "dmask": dmask, "rmask": rmask})
    return in_maps


def kernel(inputs, W):
    from concourse.bass_utils import run_bass_kernel_spmd

    inputs = np.asarray(inputs, dtype=np.float32)
    W = np.asarray(W, dtype=np.float32)

    if "nc" not in _CACHE:
        _CACHE["nc"] = _build_nc()
    nc = _CACHE["nc"]

    in_maps = _host_prep(inputs, W)
    res = run_bass_kernel_spmd(nc, in_maps, core_ids=list(range(CORES)))
    outs = [res.results[c]["out"].reshape(BL, DO, N).transpose(0, 2, 1)
            for c in range(CORES)]
    return np.concatenate(outs, axis=0).astype(np.float32)

